# revision 41
# baseline (speedup 1.0000x reference)
"""Trainium2 Bass kernel for an inverse-distance-weighting (AIDW) layer.

    out[b,s,o] = sum_n features[b,s,n] * scores[b,n] * linear[n,o]
    scores[b,n] = where(mask, d2^-1, 0) / sum_n' where(mask, d2^-1, 0)   (BETA=2)

Sharding: pure data parallel over 8 NeuronCores — 4 batch elements per core,
linear weight replicated. Per core, per batch: fold scores into the weight
(Wb = scores_b[:,None] * linear, duplicated onto partitions 0:64 and 64:128),
then stream features through TensorE. Features arrive (s, n) so each pass
transposes a (128, 2x64) stage tile into two K=64 lhsT tiles (partitions 0:64
and 64:128), then runs two row-tiled concurrent K=64 fp32 matmuls producing
256 output rows per pass.
"""

import os

import numpy as np

import concourse.bass as bass
import concourse.tile as tile
from concourse import bacc, mybir
from concourse.bass_utils import run_bass_kernel_spmd
from concourse.masks import make_identity

B, S, N, O = 32, 8192, 64, 128
N_CORES = 8
BPC = B // N_CORES        # batch elements per core
PAIR = 256                # s-rows handled per transpose+matmul pass
T = S // PAIR             # passes per batch element
SC_W = 3 * N + 2          # packed score-input width: src_x | src_y | mask | tar_x | tar_y
F32 = mybir.dt.float32
BF16 = mybir.dt.bfloat16

LAST_EXEC_TIME_NS = None
_compiled = None


def _build(s=S, debug=False, pair_mode="tilepos", big_dma=True, dup_via="dma",
           compute_dtype=BF16, load_mode="hwdge_f32", chunk=8):
    CD = compute_dtype
    T = s // PAIR
    CH = min(chunk, T)         # pair-passes per DMA chunk
    assert T % CH == 0
    NCHUNK = T // CH
    nc = bacc.Bacc("TRN2", debug=debug, target_bir_lowering=False,
                   num_devices=N_CORES)
    feat = nc.dram_tensor("features", [BPC, s, N], F32, kind="ExternalInput")
    sc_in = nc.dram_tensor("score_in", [BPC, SC_W], F32, kind="ExternalInput")
    lin_dup = nc.dram_tensor("linear_dup", [2 * N, O], F32, kind="ExternalInput")
    ident_in = nc.dram_tensor("ident", [128, 128], F32, kind="ExternalInput")
    out = nc.dram_tensor("out", [BPC, s, O], F32, kind="ExternalOutput")

    with tile.TileContext(nc) as tc:
        with (
            tc.tile_pool(name="const", bufs=1) as const_pool,
            tc.tile_pool(name="sc", bufs=1) as sc_pool,
            tc.tile_pool(name="feat", bufs=4) as feat_pool,
            tc.tile_pool(name="featT", bufs=3) as featT_pool,
            tc.tile_pool(name="osb", bufs=4) as out_pool,
            tc.tile_pool(name="psT", bufs=3, space="PSUM") as psT_pool,
            tc.tile_pool(name="psO", bufs=2, space="PSUM") as psO_pool,
        ):
            ident = const_pool.tile([128, 128], F32)
            nc.scalar.dma_start(ident[:], ident_in[:, :])
            if load_mode == "hwdge_f32" or CD == F32:
                ident_c = ident
            else:
                ident_c = const_pool.tile([128, 128], CD)
                make_identity(nc, ident_c[:])
            lin_sb = const_pool.tile([2 * N, O], F32)
            nc.scalar.dma_start(lin_sb[:], lin_dup[:, :])

            # ---- scores: (BPC partitions, N stations on free dim) ----
            s_in = sc_pool.tile([BPC, SC_W], F32)
            nc.scalar.dma_start(s_in[:], sc_in[:, :])
            dx = sc_pool.tile([BPC, N], F32)
            nc.vector.tensor_scalar_sub(dx[:], s_in[:, 0:N], s_in[:, 3 * N:3 * N + 1])
            dy = sc_pool.tile([BPC, N], F32)
            nc.vector.tensor_scalar_sub(dy[:], s_in[:, N:2 * N],
                                        s_in[:, 3 * N + 1:3 * N + 2])
            dx2 = sc_pool.tile([BPC, N], F32)
            nc.vector.tensor_mul(dx2[:], dx[:], dx[:])
            dy2 = sc_pool.tile([BPC, N], F32)
            nc.vector.tensor_mul(dy2[:], dy[:], dy[:])
            d2 = sc_pool.tile([BPC, N], F32)
            nc.vector.tensor_add(d2[:], dx2[:], dy2[:])
            recip = sc_pool.tile([BPC, N], F32)
            nc.vector.reciprocal(recip[:], d2[:])
            raw = sc_pool.tile([BPC, N], F32)
            nc.vector.tensor_mul(raw[:], recip[:], s_in[:, 2 * N:3 * N])
            den = sc_pool.tile([BPC, 1], F32)
            nc.vector.reduce_sum(den[:], raw[:], axis=mybir.AxisListType.X)
            rden = sc_pool.tile([BPC, 1], F32)
            nc.vector.reciprocal(rden[:], den[:])
            scores32 = sc_pool.tile([32, N], F32)
            nc.vector.memset(scores32[:], 0.0)
            nc.vector.tensor_scalar_mul(scores32[0:BPC, :], raw[:],
                                        rden[:, 0:1])

            # ---- scores -> (N, batch), duplicated onto partitions 64:128,
            # via DVE 32x32 block transposes (keeps the PE queue free of the
            # scores dependency chain) ----
            scT = sc_pool.tile([128, 32], F32)
            for du in range(2):
                for j in range(2):
                    p0 = du * 64 + 32 * j
                    nc.vector.transpose(scT[p0:p0 + 32, 0:32],
                                        scores32[0:32, 32 * j:32 * j + 32])

            # ---- per-batch folded weights Wb = scores_b[:,None] * linear ----
            wb = const_pool.tile([128, BPC * O], F32)
            for b in range(BPC):
                nc.vector.tensor_scalar_mul(wb[:, b * O:(b + 1) * O], lin_sb[:],
                                            scT[:, b:b + 1])
            if CD == F32:
                wb_c = wb
            else:
                wb_c = const_pool.tile([128, BPC * O], CD)
                nc.vector.tensor_copy(wb_c[:], wb[:])

            # ---- main loop ----
            # s-row decomposition: s = (t*128 + p)*8 + j, j in 0..7 —
            # partition p holds EIGHT consecutive rows per t-slice (1024
            # rows), so DMA chunks are 2KB (in) and 4KB (out) contiguous.
            # Each t-slice takes four transposes (j-pairs {2q, 2q+1});
            # transpose row-group 0:64 holds the even j, 64:128 the odd j.
            load_dtype = F32 if load_mode == "hwdge_f32" else CD
            TQ = s // 1024             # t-slices per batch
            CHT = min(2, TQ)           # t-slices per DMA chunk
            assert TQ % CHT == 0
            for b in range(BPC):
                fv = feat[b].rearrange("(t p j) n -> p t j n", j=8, p=128)
                ov = out[b].rearrange("(t p j) o -> p t j o", j=8, p=128)
                for c in range(TQ // CHT):
                    f_sb = feat_pool.tile([128, CHT, 8, N], load_dtype)
                    if load_mode == "hwdge_f32":
                        nc.sync.dma_start(f_sb[:], fv[:, c * CHT:(c + 1) * CHT])
                    else:
                        nc.gpsimd.dma_start(f_sb[:],
                                            fv[:, c * CHT:(c + 1) * CHT])
                    o_sb = out_pool.tile([128, CHT, 8, O], F32)
                    # Per t-slice: 4 transposes into one psT bank, ONE fT
                    # copy, 8 back-to-back matmuls into a psA and a psB bank
                    # (concurrent row-group pairs must hit DIFFERENT banks —
                    # same-bank concurrent PE writes take the device down),
                    # one strided copy per output bank.
                    for g in range(CHT):
                        psT = psT_pool.tile([128, 4, 128], load_dtype,
                                            tag="psT")
                        for q in range(4):
                            nc.tensor.transpose(
                                psT[:, q], f_sb[:, g, 2 * q:2 * q + 2],
                                ident if load_dtype == F32 else ident_c)
                        fT = featT_pool.tile([128, 4, 128], CD)
                        nc.scalar.copy(fT[:], psT[:])
                        # psA slot q holds rows j=2q (even); psB j=2q+1 (odd).
                        psA = psO_pool.tile([128, 4 * O], F32, tag="psA")
                        psB = psO_pool.tile([128, 4 * O], F32, tag="psB")
                        for q in range(4):
                            nc.tensor.matmul(psA[:, q * O:(q + 1) * O],
                                             fT[0:N, q, :],
                                             wb_c[0:N, b * O:(b + 1) * O],
                                             start=True, stop=True)
                            nc.tensor.matmul(psB[:, q * O:(q + 1) * O],
                                             fT[N:128, q, :],
                                             wb_c[N:128, b * O:(b + 1) * O],
                                             start=True, stop=True)
                        nc.vector.tensor_copy(o_sb[:, g, 0:8:2], psA[:])
                        nc.vector.tensor_copy(o_sb[:, g, 1:8:2], psB[:])
                    nc.scalar.dma_start(ov[:, c * CHT:(c + 1) * CHT], o_sb[:])

    nc.compile()
    return nc


def kernel(features, src_locs, tar_loc, src_masks, linear):
    global _compiled, LAST_EXEC_TIME_NS
    if _compiled is None:
        _compiled = _build()
    nc = _compiled

    features = np.asarray(features, dtype=np.float32).reshape(N_CORES, BPC, S, N)
    src_locs = np.asarray(src_locs, dtype=np.float32).reshape(N_CORES, BPC, N, 2)
    tar_loc = np.asarray(tar_loc, dtype=np.float32).reshape(N_CORES, BPC, 2)
    masks = np.asarray(src_masks).astype(np.float32).reshape(N_CORES, BPC, N)
    lin = np.asarray(linear, dtype=np.float32)
    lin_dup = np.ascontiguousarray(np.concatenate([lin, lin], axis=0))

    in_maps = []
    for i in range(N_CORES):
        sc = np.empty((BPC, SC_W), np.float32)
        sc[:, 0:N] = src_locs[i, :, :, 0]
        sc[:, N:2 * N] = src_locs[i, :, :, 1]
        sc[:, 2 * N:3 * N] = masks[i]
        sc[:, 3 * N] = tar_loc[i, :, 0]
        sc[:, 3 * N + 1] = tar_loc[i, :, 1]
        in_maps.append({
            "features": np.ascontiguousarray(features[i]),
            "score_in": sc,
            "linear_dup": lin_dup,
            "ident": np.eye(128, dtype=np.float32),
        })

    kwargs = {}
    if os.environ.get("BASS_KERNEL_TRACE", "0") == "1":
        kwargs.update(trace=True, trace_cores=[0])
        tdir = os.environ.get("BASS_KERNEL_TRACE_DIR")
        if tdir:
            os.makedirs(tdir, exist_ok=True)
            kwargs.update(tmpdir=tdir)
    res = run_bass_kernel_spmd(nc, in_maps, core_ids=list(range(N_CORES)),
                               **kwargs)
    LAST_EXEC_TIME_NS = res.exec_time_ns
    return np.concatenate([r["out"] for r in res.results], axis=0)


# revision 42
# speedup vs baseline: 1.1938x; 1.1938x over previous
"""Trainium2 Bass kernel for an inverse-distance-weighting (AIDW) layer.

    out[b,s,o] = sum_n features[b,s,n] * scores[b,n] * linear[n,o]
    scores[b,n] = where(mask, d2^-1, 0) / sum_n' where(mask, d2^-1, 0)   (BETA=2)

Sharding: pure data parallel over 8 NeuronCores — 4 batch elements per core,
linear weight replicated. Per core, per batch: fold scores into the weight
(Wb = scores_b[:,None] * linear, duplicated onto partitions 0:64 and 64:128),
then stream features through TensorE. Features arrive (s, n) so each pass
transposes a (128, 2x64) stage tile into two K=64 lhsT tiles (partitions 0:64
and 64:128), then runs two row-tiled concurrent K=64 fp32 matmuls producing
256 output rows per pass.
"""

import os

import numpy as np

import concourse.bass as bass
import concourse.tile as tile
from concourse import bacc, mybir
from concourse.bass_utils import run_bass_kernel_spmd
from concourse.masks import make_identity

B, S, N, O = 32, 8192, 64, 128
N_CORES = 8
BPC = B // N_CORES        # batch elements per core
PAIR = 256                # s-rows handled per transpose+matmul pass
T = S // PAIR             # passes per batch element
SC_W = 3 * N + 2          # packed score-input width: src_x | src_y | mask | tar_x | tar_y
F32 = mybir.dt.float32
BF16 = mybir.dt.bfloat16

LAST_EXEC_TIME_NS = None
_compiled = None


def _build(s=S, debug=False, pair_mode="tilepos", big_dma=True, dup_via="dma",
           compute_dtype=BF16, load_mode="hwdge_f32", chunk=8):
    CD = compute_dtype
    T = s // PAIR
    CH = min(chunk, T)         # pair-passes per DMA chunk
    assert T % CH == 0
    NCHUNK = T // CH
    nc = bacc.Bacc("TRN2", debug=debug, target_bir_lowering=False,
                   num_devices=N_CORES)
    feat = nc.dram_tensor("features", [BPC, s, N], F32, kind="ExternalInput")
    sc_in = nc.dram_tensor("score_in", [BPC, SC_W], F32, kind="ExternalInput")
    lin_dup = nc.dram_tensor("linear_dup", [2 * N, O], F32, kind="ExternalInput")
    ident_in = nc.dram_tensor("ident", [128, 128], F32, kind="ExternalInput")
    out = nc.dram_tensor("out", [BPC, s, O], F32, kind="ExternalOutput")

    with tile.TileContext(nc) as tc:
        with (
            tc.tile_pool(name="const", bufs=1) as const_pool,
            tc.tile_pool(name="sc", bufs=1) as sc_pool,
            tc.tile_pool(name="feat", bufs=4) as feat_pool,
            tc.tile_pool(name="featT", bufs=3) as featT_pool,
            tc.tile_pool(name="osb", bufs=4) as out_pool,
            tc.tile_pool(name="psT", bufs=2, space="PSUM") as psT_pool,
            tc.tile_pool(name="psO", bufs=2, space="PSUM") as psO_pool,
        ):
            ident = const_pool.tile([128, 128], F32)
            nc.scalar.dma_start(ident[:], ident_in[:, :])
            if load_mode == "hwdge_f32" or CD == F32:
                ident_c = ident
            else:
                ident_c = const_pool.tile([128, 128], CD)
                make_identity(nc, ident_c[:])
            lin_sb = const_pool.tile([2 * N, O], F32)
            nc.scalar.dma_start(lin_sb[:], lin_dup[:, :])

            # ---- scores: (BPC partitions, N stations on free dim) ----
            s_in = sc_pool.tile([BPC, SC_W], F32)
            nc.scalar.dma_start(s_in[:], sc_in[:, :])
            dx = sc_pool.tile([BPC, N], F32)
            nc.vector.tensor_scalar_sub(dx[:], s_in[:, 0:N], s_in[:, 3 * N:3 * N + 1])
            dy = sc_pool.tile([BPC, N], F32)
            nc.vector.tensor_scalar_sub(dy[:], s_in[:, N:2 * N],
                                        s_in[:, 3 * N + 1:3 * N + 2])
            dx2 = sc_pool.tile([BPC, N], F32)
            nc.vector.tensor_mul(dx2[:], dx[:], dx[:])
            dy2 = sc_pool.tile([BPC, N], F32)
            nc.vector.tensor_mul(dy2[:], dy[:], dy[:])
            d2 = sc_pool.tile([BPC, N], F32)
            nc.vector.tensor_add(d2[:], dx2[:], dy2[:])
            recip = sc_pool.tile([BPC, N], F32)
            nc.vector.reciprocal(recip[:], d2[:])
            raw = sc_pool.tile([BPC, N], F32)
            nc.vector.tensor_mul(raw[:], recip[:], s_in[:, 2 * N:3 * N])
            den = sc_pool.tile([BPC, 1], F32)
            nc.vector.reduce_sum(den[:], raw[:], axis=mybir.AxisListType.X)
            rden = sc_pool.tile([BPC, 1], F32)
            nc.vector.reciprocal(rden[:], den[:])
            scores32 = sc_pool.tile([32, N], F32)
            nc.vector.memset(scores32[:], 0.0)
            nc.vector.tensor_scalar_mul(scores32[0:BPC, :], raw[:],
                                        rden[:, 0:1])

            # ---- scores -> (N, batch), duplicated onto partitions 64:128,
            # via DVE 32x32 block transposes (keeps the PE queue free of the
            # scores dependency chain) ----
            scT = sc_pool.tile([128, 32], F32)
            for du in range(2):
                for j in range(2):
                    p0 = du * 64 + 32 * j
                    nc.vector.transpose(scT[p0:p0 + 32, 0:32],
                                        scores32[0:32, 32 * j:32 * j + 32])

            # ---- per-batch folded weights Wb = scores_b[:,None] * linear ----
            wb = const_pool.tile([128, BPC * O], F32)
            for b in range(BPC):
                nc.vector.tensor_scalar_mul(wb[:, b * O:(b + 1) * O], lin_sb[:],
                                            scT[:, b:b + 1])
            if CD == F32:
                wb_c = wb
            else:
                wb_c = const_pool.tile([128, BPC * O], CD)
                nc.vector.tensor_copy(wb_c[:], wb[:])

            # ---- main loop ----
            # s-row decomposition: s = (t*128 + p)*8 + j, j in 0..7 —
            # partition p holds EIGHT consecutive rows per t-slice (1024
            # rows), so DMA chunks are 2KB (in) and 4KB (out) contiguous.
            # Each t-slice takes four transposes (j-pairs {2q, 2q+1});
            # transpose row-group 0:64 holds the even j, 64:128 the odd j.
            load_dtype = F32 if load_mode == "hwdge_f32" else CD
            TQ = s // 1024             # t-slices per batch
            CHT = min(2, TQ)           # t-slices per DMA chunk
            assert TQ % CHT == 0
            for b in range(BPC):
                fv = feat[b].rearrange("(t p j) n -> p t j n", j=8, p=128)
                ov = out[b].rearrange("(t p j) o -> p t j o", j=8, p=128)
                for c in range(TQ // CHT):
                    f_sb = feat_pool.tile([128, CHT, 8, N], load_dtype)
                    if load_mode == "hwdge_f32":
                        nc.sync.dma_start(f_sb[:], fv[:, c * CHT:(c + 1) * CHT])
                    else:
                        nc.gpsimd.dma_start(f_sb[:],
                                            fv[:, c * CHT:(c + 1) * CHT])
                    o_sb = out_pool.tile([128, CHT, 8, O], F32)
                    # Per t-slice: 4 transposes into one psT bank, ONE fT
                    # copy, 8 back-to-back matmuls into a psA and a psB bank
                    # (concurrent row-group pairs must hit DIFFERENT banks —
                    # same-bank concurrent PE writes take the device down),
                    # one strided copy per output bank.
                    for g in range(CHT):
                        psT = psT_pool.tile([128, 4, 128], load_dtype,
                                            tag="psT")
                        for q in range(4):
                            nc.tensor.transpose(
                                psT[:, q], f_sb[:, g, 2 * q:2 * q + 2],
                                ident if load_dtype == F32 else ident_c)
                        fT = featT_pool.tile([128, 4, 128], CD)
                        nc.scalar.copy(fT[:], psT[:])
                        # psA slot q holds rows j=2q (even); psB j=2q+1 (odd).
                        psA = psO_pool.tile([128, 4 * O], F32, tag="psA")
                        psB = psO_pool.tile([128, 4 * O], F32, tag="psB")
                        for q in range(4):
                            nc.tensor.matmul(psA[:, q * O:(q + 1) * O],
                                             fT[0:N, q, :],
                                             wb_c[0:N, b * O:(b + 1) * O],
                                             start=True, stop=True)
                            nc.tensor.matmul(psB[:, q * O:(q + 1) * O],
                                             fT[N:128, q, :],
                                             wb_c[N:128, b * O:(b + 1) * O],
                                             start=True, stop=True)
                        nc.vector.tensor_copy(o_sb[:, g, 0:8:2], psA[:])
                        nc.vector.tensor_copy(o_sb[:, g, 1:8:2], psB[:])
                    nc.scalar.dma_start(ov[:, c * CHT:(c + 1) * CHT], o_sb[:])

    nc.compile()
    return nc


def kernel(features, src_locs, tar_loc, src_masks, linear):
    global _compiled, LAST_EXEC_TIME_NS
    if _compiled is None:
        _compiled = _build()
    nc = _compiled

    features = np.asarray(features, dtype=np.float32).reshape(N_CORES, BPC, S, N)
    src_locs = np.asarray(src_locs, dtype=np.float32).reshape(N_CORES, BPC, N, 2)
    tar_loc = np.asarray(tar_loc, dtype=np.float32).reshape(N_CORES, BPC, 2)
    masks = np.asarray(src_masks).astype(np.float32).reshape(N_CORES, BPC, N)
    lin = np.asarray(linear, dtype=np.float32)
    lin_dup = np.ascontiguousarray(np.concatenate([lin, lin], axis=0))

    in_maps = []
    for i in range(N_CORES):
        sc = np.empty((BPC, SC_W), np.float32)
        sc[:, 0:N] = src_locs[i, :, :, 0]
        sc[:, N:2 * N] = src_locs[i, :, :, 1]
        sc[:, 2 * N:3 * N] = masks[i]
        sc[:, 3 * N] = tar_loc[i, :, 0]
        sc[:, 3 * N + 1] = tar_loc[i, :, 1]
        in_maps.append({
            "features": np.ascontiguousarray(features[i]),
            "score_in": sc,
            "linear_dup": lin_dup,
            "ident": np.eye(128, dtype=np.float32),
        })

    kwargs = {}
    if os.environ.get("BASS_KERNEL_TRACE", "0") == "1":
        kwargs.update(trace=True, trace_cores=[0])
        tdir = os.environ.get("BASS_KERNEL_TRACE_DIR")
        if tdir:
            os.makedirs(tdir, exist_ok=True)
            kwargs.update(tmpdir=tdir)
    res = run_bass_kernel_spmd(nc, in_maps, core_ids=list(range(N_CORES)),
                               **kwargs)
    LAST_EXEC_TIME_NS = res.exec_time_ns
    return np.concatenate([r["out"] for r in res.results], axis=0)
